# revision 23
# baseline (speedup 1.0000x reference)
"""Diagonalizable linear plant (modal state-space scan) on 8 Trainium2 cores.

y[b,t] = Cz @ z[b,t-1] + D @ u[b,t],  z[b,t] = lam * z[b,t-1] + Bz @ u[b,t]
with z[b,-1] = z0[b] = x0[b] @ Q, Bz = Q^T Bmat, Cz = C Q.

Sharding: data-parallel over batch (16 batches -> 2 per core).

Block-8 formulation (the DVE scan instruction runs at ~2 cycles/element,
so the time axis is decimated 8x before it reaches the scan; everything
else is full 128x128xN=512 bf16 matmuls, fp32 PSUM):
  host packs u as uT8[(i*32+u), k] = u[8k+i, u]        (256 rows = 2 K-groups)
  PE   V_h = W2^T @ U          W2[(i,u),n] = lam_n^(7-i) Bz[n,u]
  DVE  zB = scan(lam^8, V)     block-boundary states z_{8k+7}
  PE   Y_g = WC^T @ zBprev + WU^T @ U     (g indexes (j,y) output groups)
       WC[n,(j,y)] = lam_n^j Cz[y,n]
       WU[(i,u),(j,y)] = (Cz lam^(j-1-i) Bz)[y,u] for i<j, D[y,u] for i=j, else 0
  host unpacks yT8[(32j+y), k] -> y[8k+j, y]
"""

import numpy as np

B, T, NX, NU, NY = 16, 8192, 256, 32, 32
NCORES = 8
BPC = B // NCORES   # batches per core
MB = 8              # time-block folded into matmul K
KCOL = T // MB      # block columns per batch (1024)
L = 512             # block-columns per chunk
NCHUNK = KCOL // L  # chunks per batch (2)

_PROG = None  # built Bass program, cached across kernel() calls


def _patch_tile_drain():
    """walrus codegen in this container rejects >1 sync wait on one SP
    TPB_CTRL instruction (terminal TileContext drain / NoOp). Split the
    drain's waits across preceding SP nops carrying one wait each."""
    import concourse.tile as tile
    import concourse.mybir as mybir
    from concourse.vector_clock import ScopedClock

    if getattr(tile.TileContext, "_drain_patched", False):
        return

    def _drain_and_barrier(self, tick_clock, wait_clock):
        nc = self.nc
        scratch = nc.sync.nop()
        wait_clock.add_sem_waits(
            scratch.ins, ScopedClock({None: tick_clock.global_clock})
        )
        si = scratch.ins.sync_info
        waits = list(si.on_wait) if si is not None else []
        scratch.ins.sync_info = mybir.SyncInfo(on_wait=waits[:1], on_update=[])
        for w in waits[1:]:
            n2 = nc.sync.nop()
            n2.ins.sync_info = mybir.SyncInfo(on_wait=[w], on_update=[])
        nc.sync.drain()
        nc.all_engine_barrier()
        assert self.sems is not None
        popped = nc._tile_sem_poison_stack.pop()
        assert popped is self._sem_poison
        nc.clear_and_free_semaphores(list(self.sems.allocated().values()))
        nc.all_engine_barrier()

    tile.TileContext._drain_and_barrier = _drain_and_barrier
    tile.TileContext._drain_patched = True


def _split_multi_waits(nc, mybir):
    """This container's walrus codegen accepts at most ONE sync wait per
    instruction. Hoist extra waits into standalone EventSemaphore nops on
    the same engine, placed immediately before the instruction."""
    ctr = [0]

    def fresh(engine, wait):
        ctr[0] += 1
        ev = mybir.InstEventSemaphore(name=f"I-wsplit-{ctr[0]}", ins=[], outs=[])
        ev.engine = engine
        ev.sync_info = mybir.SyncInfo(on_wait=[wait], on_update=[])
        nc.register_instruction(ev)
        return ev

    for fn in nc.m.functions:
        for bb in fn.blocks:
            out = []
            changed = False
            for inst in bb.instructions:
                si = inst.sync_info
                waits = list(si.on_wait) if si is not None else []
                if len(waits) > 1:
                    changed = True
                    for w in waits[:-1]:
                        out.append(fresh(inst.engine, w))
                    inst.sync_info = mybir.SyncInfo(
                        on_wait=[waits[-1]], on_update=list(si.on_update)
                    )
                out.append(inst)
            if changed:
                bb.instructions = out


def build_program():
    import concourse.bass as bass
    import concourse.tile as tile
    import concourse.mybir as mybir
    from contextlib import ExitStack

    _patch_tile_drain()
    f32 = mybir.dt.float32
    bf = mybir.dt.bfloat16

    nc = bass.Bass()
    QL = L // 2  # U-load quarter (256 block-cols)
    # uT8q[b, q, (i*32+u), k] = u-block column q*QL+k; each (b, g, q) DMA
    # reads a fully contiguous 64 KB region (4 KB packets)
    uT8q = nc.declare_dram_parameter("uT8q", [BPC, 4, 256, QL], bf, isOutput=False)
    wAll = nc.declare_dram_parameter("wAll", [128, 11 * 128], bf, isOutput=False)
    # pz: col 0:2 = lam^8 halves, cols 2: = z0 modal states (merged tiny DMA)
    pz = nc.declare_dram_parameter("pz", [128, 2 + 2 * BPC], f32, isOutput=False)
    yT8s = nc.declare_dram_parameter("yT8s", [BPC, 2, 256, L], bf, isOutput=True)

    with ExitStack() as ctx:
        tc = ctx.enter_context(tile.TileContext(nc))
        const = ctx.enter_context(tc.tile_pool(name="const", bufs=1))
        vps = ctx.enter_context(tc.tile_pool(name="vps", bufs=2, space="PSUM"))
        yps = ctx.enter_context(tc.tile_pool(name="yps", bufs=2, space="PSUM"))
        zpool = ctx.enter_context(tc.tile_pool(name="z", bufs=6))
        yout = ctx.enter_context(tc.tile_pool(name="yo", bufs=4))

        # DMA plan: the two HWDGE queues (scalar=Activation, sync=SP) carry
        # all bulk traffic, balanced so the first unit's operands (W2 for
        # h0+h1, U halves for b0) land earliest on both queues. The slow
        # software gpsimd queue gets only the Y-phase weights, which
        # stream in the background and aren't needed until ~+6us.
        W2t = const.tile([128, 512], bf)
        pzt = const.tile([128, 2 + 2 * BPC], f32)
        nc.sync.dma_start(pzt[:], pz[:])
        # weights ride the gpsimd software queue in priority order (W2 →
        # WC → WU), leaving both HWDGE queues free for pure U streaming
        nc.gpsimd.dma_start(W2t[:], wAll[:, 0:512])
        WCt = const.tile([128, 512], bf)
        nc.gpsimd.dma_start(WCt[:], wAll[:, 512:1024])
        WUt = const.tile([128, 384], bf)
        nc.gpsimd.dma_start(WUt[:], wAll[:, 1024:1408])
        lam8t = pzt[:, 0:2]
        z0t = pzt[:, 2 : 2 + 2 * BPC]
        # U tiles: [b][g] -> [128, KCOL]; quarter-column DMAs issued in unit
        # consumption order so the first scan only waits on 64 KB per queue
        Ubig = [[const.tile([128, KCOL], bf, name=f"U{g}b{b}") for g in range(2)]
                for b in range(BPC)]
        for q in range(4):
            for b in range(BPC):
                sl = slice(q * QL, (q + 1) * QL)
                nc.scalar.dma_start(Ubig[b][0][:, sl], uT8q[b, q, 0:128, :])
                nc.sync.dma_start(Ubig[b][1][:, sl], uT8q[b, q, 128:256, :])

        # PE warm-up matmuls fill the whole DMA window back-to-back so the
        # clock governor sees sustained activity before real work begins
        dummy = const.tile([128, L], bf)
        nc.vector.memset(dummy[:], 0.0)
        WP = vps.tile([128, L], f32, name="WP", tag="V0")
        for _ in range(3):
            nc.tensor.matmul(WP[:], lhsT=dummy[:, 0:128], rhs=dummy[:],
                             start=True, stop=True)

        # lam broadcast built on DVE during the DMA fill window
        ones = const.tile([128, L], f32)
        nc.vector.memset(ones[:], 1.0)
        lam_bc = const.tile([128, 2 * L], f32)
        for h in range(2):
            nc.vector.tensor_scalar_mul(
                lam_bc[:, h * L : (h + 1) * L], ones[:], lam8t[:, h : h + 1]
            )

        def w2blk(i):
            return W2t[:, 128 * i : 128 * (i + 1)]

        def wcblk(i):
            return WCt[:, 128 * i : 128 * (i + 1)]

        # wAll W2 block order: [g0h0, g1h0, g0h1, g1h1] (h=0 pair first)
        W2 = [[w2blk(0), w2blk(2)], [w2blk(1), w2blk(3)]]      # [g][h]
        WC = [[wcblk(0), wcblk(1)], [wcblk(2), wcblk(3)]]      # [h][g]
        WU00 = WUt[:, 0:128]
        WU01 = WUt[:, 128:256]
        WU11 = WUt[:, 256:384]                                 # WU[1][0] == 0

        mult = mybir.AluOpType.mult
        add = mybir.AluOpType.add

        prev_z = [[None, None] for _ in range(BPC)]

        def emit_vscan(c, b):
            sl = slice(c * L, (c + 1) * L)
            U = [Ubig[b][0][:, sl], Ubig[b][1][:, sl]]
            nq = 2                   # sub-chunks per V/scan (quarters)
            SL = L // nq
            zext = [None, None]
            for h in range(2):
                V = vps.tile([128, L], f32, name=f"V{h}_{b}_{c}", tag=f"V{h}")
                Z = zpool.tile([128, L + 1], bf, name=f"Z{h}_{b}_{c}",
                               tag=f"Z{h}")
                if c == 0:
                    carry = z0t[:, 2 * b + h : 2 * b + h + 1]
                else:
                    carry = prev_z[b][h][:, L : L + 1]
                for s in range(nq):
                    ss = slice(s * SL, (s + 1) * SL)
                    nc.tensor.matmul(V[:, ss], lhsT=W2[0][h], rhs=U[0][:, ss],
                                     start=True, stop=False)
                    nc.tensor.matmul(V[:, ss], lhsT=W2[1][h], rhs=U[1][:, ss],
                                     start=False, stop=True)
                    nc.vector.tensor_tensor_scan(
                        Z[:, 1 + s * SL : 1 + (s + 1) * SL],
                        lam_bc[:, h * L : h * L + SL], V[:, ss],
                        carry, mult, add,
                    )
                    carry = Z[:, (s + 1) * SL : (s + 1) * SL + 1]
                nc.gpsimd.tensor_copy(
                    Z[:, 0:1],
                    z0t[:, 2 * b + h : 2 * b + h + 1] if c == 0
                    else prev_z[b][h][:, L : L + 1])
                zext[h] = Z
            prev_z[b] = zext
            return U, zext

        def emit_y(c, b, U, zext, last=False):
            for g in range(2):
                Y = yps.tile([128, L], f32, name=f"Y{g}_{b}_{c}", tag=f"Y{g}")
                nc.tensor.matmul(Y[:], lhsT=WC[0][g], rhs=zext[0][:, 0:L],
                                 start=True, stop=False)
                nc.tensor.matmul(Y[:], lhsT=WC[1][g], rhs=zext[1][:, 0:L],
                                 start=False, stop=False)
                if g == 0:
                    nc.tensor.matmul(Y[:], lhsT=WU00, rhs=U[0],
                                     start=False, stop=True)
                else:
                    nc.tensor.matmul(Y[:], lhsT=WU01, rhs=U[0],
                                     start=False, stop=False)
                    nc.tensor.matmul(Y[:], lhsT=WU11, rhs=U[1],
                                     start=False, stop=True)
                Ysb = yout.tile([128, L], bf, name=f"Ysb{g}_{b}_{c}",
                                tag=f"Ysb{g}")
                oeng = nc.sync if g == 0 else nc.scalar
                dst = yT8s[b, c, 128 * g : 128 * (g + 1), :]
                if not last:
                    nc.scalar.copy(Ysb[:], Y[:])
                    oeng.dma_start(dst, Ysb[:])
                else:
                    # tail: copies split in halves and spread over vector
                    # (g0, free after the last scan) and scalar (g1) so the
                    # final stores issue as early as possible
                    H = L // 2
                    for p in range(2):
                        s = slice(p * H, (p + 1) * H)
                        if g == 0:
                            nc.vector.tensor_copy(Ysb[:, s], Y[:, s])
                        else:
                            nc.scalar.copy(Ysb[:, s], Y[:, s])
                        oeng.dma_start(dst[:, s], Ysb[:, s])

        units = [(c, b) for c in range(NCHUNK) for b in range(BPC)]
        pending = []
        for (c, b) in units:
            U, zext = emit_vscan(c, b)
            pending.append((c, b, U, zext))
            if len(pending) > 2:
                emit_y(*pending.pop(0))
        for i, p in enumerate(pending):
            emit_y(*p, last=(i == len(pending) - 1))

    _split_multi_waits(nc, mybir)
    return nc


def _host_prep(x0, u, Q, lam, Bmat, C, D):
    import ml_dtypes

    f = np.float32
    bfd = ml_dtypes.bfloat16
    lam = lam.astype(f)
    Bz = (Q.T.astype(f) @ Bmat.astype(f)).astype(f)      # (NX, NU)
    Cz = (C.astype(f) @ Q.astype(f)).astype(f)           # (NY, NX)
    z0 = (x0.astype(f) @ Q.astype(f)).astype(f)          # (B, NX)

    lam_p = np.stack([lam**j for j in range(MB)])         # (MB, NX)

    # W2[(i*32+u), n] = lam_n^(MB-1-i) * Bz[n, u]
    W2 = np.einsum("in,nu->iun", lam_p[::-1], Bz).reshape(MB * NU, NX)
    # WC[n, (32j+y)] = lam_n^j * Cz[y, n]
    WC = np.einsum("jn,yn->njy", lam_p, Cz).reshape(NX, MB * NY)
    # WU[(i*32+u), (32j+y)]
    WU = np.zeros((MB * NU, MB * NY), dtype=f)
    for j in range(MB):
        for i in range(MB):
            if i < j:
                Mji = (Cz * lam_p[j - 1 - i][None, :]) @ Bz   # (NY, NU)
                WU[i * NU : (i + 1) * NU, j * NY : (j + 1) * NY] = Mji.T
            elif i == j:
                WU[i * NU : (i + 1) * NU, j * NY : (j + 1) * NY] = D.T.astype(f)

    blocks = []
    for h in range(2):          # W2 order [g0h0, g1h0, g0h1, g1h1]
        for g in range(2):
            blocks.append(W2[128 * g : 128 * (g + 1), 128 * h : 128 * (h + 1)])
    for h in range(2):          # WC[h][g]
        for g in range(2):
            blocks.append(WC[128 * h : 128 * (h + 1), 128 * g : 128 * (g + 1)])
    # WU[g2][g] blocks; WU[1][0] is identically zero (i > j) and skipped
    blocks.append(WU[0:128, 0:128])      # WU00
    blocks.append(WU[0:128, 128:256])    # WU01
    blocks.append(WU[128:256, 128:256])  # WU11
    wAll = np.concatenate(blocks, axis=1).astype(bfd)     # (128, 11*128)

    # uT8q[b, q, (i*32+u), k] = u[b, 8*(q*QL+k)+i, u]  (column-quarter major)
    QL = L // 2
    uT8 = u.reshape(B, KCOL, MB, NU).transpose(0, 2, 3, 1).reshape(B, MB * NU, KCOL)
    uT8q = np.ascontiguousarray(
        uT8.reshape(B, MB * NU, 4, QL).transpose(0, 2, 1, 3)
    ).astype(bfd)

    lam8 = lam**MB
    lam8c = np.stack([lam8[:128], lam8[128:]], axis=1).astype(f)  # (128, 2)
    return wAll, z0, uT8q, lam8c


def make_in_maps(x0, u, Q, lam, Bmat, C, D):
    wAll, z0, uT8q, lam8c = _host_prep(x0, u, Q, lam, Bmat, C, D)
    in_maps = []
    for cidx in range(NCORES):
        sl = slice(cidx * BPC, (cidx + 1) * BPC)
        z0_c = z0[sl]
        z0c = z0_c.reshape(BPC, 2, 128).transpose(2, 0, 1).reshape(128, 2 * BPC)
        pz = np.ascontiguousarray(np.concatenate([lam8c, z0c], axis=1))
        in_maps.append(
            {
                "uT8q": np.ascontiguousarray(uT8q[sl]),
                "wAll": wAll,
                "pz": pz,
            }
        )
    return in_maps


def kernel(x0, u, Q, lam, Bmat, C, D):
    global _PROG
    from concourse.bass_utils import run_bass_kernel_spmd

    if _PROG is None:
        _PROG = build_program()
    in_maps = make_in_maps(x0, u, Q, lam, Bmat, C, D)
    res = run_bass_kernel_spmd(_PROG, in_maps, list(range(NCORES)))
    y = np.empty((B, T, NY), dtype=np.float32)
    for cidx in range(NCORES):
        yT8s_c = res.results[cidx]["yT8s"].astype(np.float32)  # (BPC, 2, 256, L)
        # y[b, 8*(ch*L+k)+j, yy] = yT8s[b, ch, 32j+yy, k]
        y[cidx * BPC : (cidx + 1) * BPC] = (
            yT8s_c.reshape(BPC, 2, MB, NY, L)
            .transpose(0, 1, 4, 2, 3)
            .reshape(BPC, T, NY)
        )
    return y



# revision 25
# speedup vs baseline: 1.0518x; 1.0518x over previous
"""Diagonalizable linear plant (modal state-space scan) on 8 Trainium2 cores.

y[b,t] = Cz @ z[b,t-1] + D @ u[b,t],  z[b,t] = lam * z[b,t-1] + Bz @ u[b,t]
with z[b,-1] = z0[b] = x0[b] @ Q, Bz = Q^T Bmat, Cz = C Q.

Sharding: data-parallel over batch (16 batches -> 2 per core).

Block-8 formulation (the DVE scan instruction runs at ~2 cycles/element,
so the time axis is decimated 8x before it reaches the scan; everything
else is full 128x128xN=512 bf16 matmuls, fp32 PSUM):
  host packs u as uT8[(i*32+u), k] = u[8k+i, u]        (256 rows = 2 K-groups)
  PE   V_h = W2^T @ U          W2[(i,u),n] = lam_n^(7-i) Bz[n,u]
  DVE  zB = scan(lam^8, V)     block-boundary states z_{8k+7}
  PE   Y_g = WC^T @ zBprev + WU^T @ U     (g indexes (j,y) output groups)
       WC[n,(j,y)] = lam_n^j Cz[y,n]
       WU[(i,u),(j,y)] = (Cz lam^(j-1-i) Bz)[y,u] for i<j, D[y,u] for i=j, else 0
  host unpacks yT8[(32j+y), k] -> y[8k+j, y]
"""

import numpy as np

B, T, NX, NU, NY = 16, 8192, 256, 32, 32
NCORES = 8
BPC = B // NCORES   # batches per core
MB = 8              # time-block folded into matmul K
KCOL = T // MB      # block columns per batch (1024)
L = 512             # block-columns per chunk
NCHUNK = KCOL // L  # chunks per batch (2)

_PROG = None  # built Bass program, cached across kernel() calls


def _patch_tile_drain():
    """walrus codegen in this container rejects >1 sync wait on one SP
    TPB_CTRL instruction (terminal TileContext drain / NoOp). Split the
    drain's waits across preceding SP nops carrying one wait each."""
    import concourse.tile as tile
    import concourse.mybir as mybir
    from concourse.vector_clock import ScopedClock

    if getattr(tile.TileContext, "_drain_patched", False):
        return

    def _drain_and_barrier(self, tick_clock, wait_clock):
        nc = self.nc
        scratch = nc.sync.nop()
        wait_clock.add_sem_waits(
            scratch.ins, ScopedClock({None: tick_clock.global_clock})
        )
        si = scratch.ins.sync_info
        waits = list(si.on_wait) if si is not None else []
        scratch.ins.sync_info = mybir.SyncInfo(on_wait=waits[:1], on_update=[])
        for w in waits[1:]:
            n2 = nc.sync.nop()
            n2.ins.sync_info = mybir.SyncInfo(on_wait=[w], on_update=[])
        nc.sync.drain()
        nc.all_engine_barrier()
        assert self.sems is not None
        popped = nc._tile_sem_poison_stack.pop()
        assert popped is self._sem_poison
        nc.clear_and_free_semaphores(list(self.sems.allocated().values()))
        nc.all_engine_barrier()

    tile.TileContext._drain_and_barrier = _drain_and_barrier
    tile.TileContext._drain_patched = True


def _split_multi_waits(nc, mybir):
    """This container's walrus codegen accepts at most ONE sync wait per
    instruction. Hoist extra waits into standalone EventSemaphore nops on
    the same engine, placed immediately before the instruction."""
    ctr = [0]

    def fresh(engine, wait):
        ctr[0] += 1
        ev = mybir.InstEventSemaphore(name=f"I-wsplit-{ctr[0]}", ins=[], outs=[])
        ev.engine = engine
        ev.sync_info = mybir.SyncInfo(on_wait=[wait], on_update=[])
        nc.register_instruction(ev)
        return ev

    for fn in nc.m.functions:
        for bb in fn.blocks:
            out = []
            changed = False
            for inst in bb.instructions:
                si = inst.sync_info
                waits = list(si.on_wait) if si is not None else []
                if len(waits) > 1:
                    changed = True
                    for w in waits[:-1]:
                        out.append(fresh(inst.engine, w))
                    inst.sync_info = mybir.SyncInfo(
                        on_wait=[waits[-1]], on_update=list(si.on_update)
                    )
                out.append(inst)
            if changed:
                bb.instructions = out


def build_program():
    import concourse.bass as bass
    import concourse.tile as tile
    import concourse.mybir as mybir
    from contextlib import ExitStack

    _patch_tile_drain()
    f32 = mybir.dt.float32
    bf = mybir.dt.bfloat16

    nc = bass.Bass()
    QL = L // 2  # 256 block-cols
    # uA[g, q, (i*32+u)%128, k]: quarters q0/q1 of batch-0 chunk-0 (64 KB
    # contiguous each) — lets the first scans start as early as possible.
    uA = nc.declare_dram_parameter("uA", [2, 2, 128, QL], bf, isOutput=False)
    # uB[j, g, row, k]: half-chunks in consumption order
    # j=0: b1 ch0, j=1: b0 ch1, j=2: b1 ch1   (128 KB contiguous each)
    uB = nc.declare_dram_parameter("uB", [3, 2, 128, L], bf, isOutput=False)
    wAll = nc.declare_dram_parameter("wAll", [128, 11 * 128], bf, isOutput=False)
    # pz: col 0:2 = lam^8 halves, cols 2: = z0 modal states (merged tiny DMA)
    pz = nc.declare_dram_parameter("pz", [128, 2 + 2 * BPC], f32, isOutput=False)
    yT8s = nc.declare_dram_parameter("yT8s", [BPC, 2, 256, L], bf, isOutput=True)

    with ExitStack() as ctx:
        tc = ctx.enter_context(tile.TileContext(nc))
        const = ctx.enter_context(tc.tile_pool(name="const", bufs=1))
        vps = ctx.enter_context(tc.tile_pool(name="vps", bufs=2, space="PSUM"))
        yps = ctx.enter_context(tc.tile_pool(name="yps", bufs=2, space="PSUM"))
        zpool = ctx.enter_context(tc.tile_pool(name="z", bufs=6))
        yout = ctx.enter_context(tc.tile_pool(name="yo", bufs=4))

        # DMA plan: scalar queue carries all g=0 (rows 0:128) U tiles,
        # sync all g=1, both in unit-consumption order; weights ride the
        # gpsimd software queue (W2 first, then WC, WU for the Y phase).
        pzt = const.tile([128, 2 + 2 * BPC], f32)
        nc.sync.dma_start(pzt[:], pz[:])
        W2t = const.tile([128, 512], bf)
        nc.gpsimd.dma_start(W2t[:], wAll[:, 0:512])
        WCt = const.tile([128, 512], bf)
        nc.gpsimd.dma_start(WCt[:], wAll[:, 512:1024])
        WUt = const.tile([128, 384], bf)
        nc.gpsimd.dma_start(WUt[:], wAll[:, 1024:1408])
        lam8t = pzt[:, 0:2]
        z0t = pzt[:, 2 : 2 + 2 * BPC]
        Ubig = [[const.tile([128, KCOL], bf, name=f"U{g}b{b}") for g in range(2)]
                for b in range(BPC)]
        qeng = [nc.scalar, nc.sync]
        for g in range(2):
            for q in range(2):                       # b0 ch0 quarters
                qeng[g].dma_start(Ubig[0][g][:, q * QL : (q + 1) * QL],
                                  uA[g, q, :, :])
        for j, (bb, ch) in enumerate([(1, 0), (0, 1), (1, 1)]):  # halves
            for g in range(2):
                qeng[g].dma_start(Ubig[bb][g][:, ch * L : (ch + 1) * L],
                                  uB[j, g, :, :])

        # PE warm-up during the DMA fill window
        dummy = const.tile([128, L], bf)
        nc.vector.memset(dummy[:], 0.0)
        WP = vps.tile([128, L], f32, name="WP", tag="V0")
        for _ in range(3):
            nc.tensor.matmul(WP[:], lhsT=dummy[:, 0:128], rhs=dummy[:],
                             start=True, stop=True)

        # lam broadcast built on DVE during the DMA fill window
        ones = const.tile([128, L], f32)
        nc.vector.memset(ones[:], 1.0)
        lam_bc = const.tile([128, 2 * L], f32)
        for h in range(2):
            nc.vector.tensor_scalar_mul(
                lam_bc[:, h * L : (h + 1) * L], ones[:], lam8t[:, h : h + 1]
            )

        def w2blk(i):
            return W2t[:, 128 * i : 128 * (i + 1)]

        def wcblk(i):
            return WCt[:, 128 * i : 128 * (i + 1)]

        # wAll W2 block order: [g0h0, g1h0, g0h1, g1h1] (h=0 pair first)
        W2 = [[w2blk(0), w2blk(2)], [w2blk(1), w2blk(3)]]      # [g][h]
        WC = [[wcblk(0), wcblk(1)], [wcblk(2), wcblk(3)]]      # [h][g]
        WU00 = WUt[:, 0:128]
        WU01 = WUt[:, 128:256]
        WU11 = WUt[:, 256:384]                                 # WU[1][0] == 0

        mult = mybir.AluOpType.mult
        add = mybir.AluOpType.add

        prev_z = [[None, None] for _ in range(BPC)]

        def emit_vscan(c, b, split=False):
            sl = slice(c * L, (c + 1) * L)
            U = [Ubig[b][0][:, sl], Ubig[b][1][:, sl]]
            nq = 2 if split else 1
            SL = L // nq
            zext = [None, None]
            for h in range(2):
                V = vps.tile([128, L], f32, name=f"V{h}_{b}_{c}", tag=f"V{h}")
                Z = zpool.tile([128, L + 1], bf, name=f"Z{h}_{b}_{c}",
                               tag=f"Z{h}")
                carry0 = (z0t[:, 2 * b + h : 2 * b + h + 1] if c == 0
                          else prev_z[b][h][:, L : L + 1])
                carry = carry0
                for s in range(nq):
                    ss = slice(s * SL, (s + 1) * SL)
                    nc.tensor.matmul(V[:, ss], lhsT=W2[0][h], rhs=U[0][:, ss],
                                     start=True, stop=False)
                    nc.tensor.matmul(V[:, ss], lhsT=W2[1][h], rhs=U[1][:, ss],
                                     start=False, stop=True)
                    nc.vector.tensor_tensor_scan(
                        Z[:, 1 + s * SL : 1 + (s + 1) * SL],
                        lam_bc[:, h * L : h * L + SL], V[:, ss],
                        carry, mult, add,
                    )
                    carry = Z[:, (s + 1) * SL : (s + 1) * SL + 1]
                nc.gpsimd.tensor_copy(Z[:, 0:1], carry0)
                zext[h] = Z
            prev_z[b] = zext
            return U, zext

        def emit_y(c, b, U, zext, last=False):
            for g in range(2):
                Y = yps.tile([128, L], f32, name=f"Y{g}_{b}_{c}", tag=f"Y{g}")
                wu = ([(WU00, U[0])] if g == 0
                      else [(WU01, U[0]), (WU11, U[1])])
                wc = [(WC[0][g], zext[0][:, 0:L]), (WC[1][g], zext[1][:, 0:L])]
                # early units: WC weights arrive before WU; late units: WU
                # is long loaded and scan h1 lands last, so WU goes first
                mms = wc + wu if c == 0 else wu + wc
                for i, (lhsT, rhs) in enumerate(mms):
                    nc.tensor.matmul(Y[:], lhsT=lhsT, rhs=rhs,
                                     start=(i == 0), stop=(i == len(mms) - 1))
                Ysb = yout.tile([128, L], bf, name=f"Ysb{g}_{b}_{c}",
                                tag=f"Ysb{g}")
                oeng = nc.sync if g == 0 else nc.scalar
                dst = yT8s[b, c, 128 * g : 128 * (g + 1), :]
                if not last:
                    nc.scalar.copy(Ysb[:], Y[:])
                    oeng.dma_start(dst, Ysb[:])
                else:
                    # tail: halves; g0 copies on vector (free after the last
                    # scan), g1 on scalar, stores fan out to both queues
                    H = L // 2
                    for p in range(2):
                        s = slice(p * H, (p + 1) * H)
                        if g == 0:
                            nc.vector.tensor_copy(Ysb[:, s], Y[:, s])
                        else:
                            nc.scalar.copy(Ysb[:, s], Y[:, s])
                        oeng.dma_start(dst[:, s], Ysb[:, s])

        units = [(c, b) for c in range(NCHUNK) for b in range(BPC)]
        pending = []
        for (c, b) in units:
            U, zext = emit_vscan(c, b, split=(c == 0 and b == 0))
            pending.append((c, b, U, zext))
            if len(pending) > 2:
                emit_y(*pending.pop(0))
        for i, p in enumerate(pending):
            emit_y(*p, last=(i == len(pending) - 1))

    _split_multi_waits(nc, mybir)
    return nc


def _host_prep(x0, u, Q, lam, Bmat, C, D):
    import ml_dtypes

    f = np.float32
    bfd = ml_dtypes.bfloat16
    lam = lam.astype(f)
    Bz = (Q.T.astype(f) @ Bmat.astype(f)).astype(f)      # (NX, NU)
    Cz = (C.astype(f) @ Q.astype(f)).astype(f)           # (NY, NX)
    z0 = (x0.astype(f) @ Q.astype(f)).astype(f)          # (B, NX)

    lam_p = np.stack([lam**j for j in range(MB)])         # (MB, NX)

    # W2[(i*32+u), n] = lam_n^(MB-1-i) * Bz[n, u]
    W2 = np.einsum("in,nu->iun", lam_p[::-1], Bz).reshape(MB * NU, NX)
    # WC[n, (32j+y)] = lam_n^j * Cz[y, n]
    WC = np.einsum("jn,yn->njy", lam_p, Cz).reshape(NX, MB * NY)
    # WU[(i*32+u), (32j+y)]
    WU = np.zeros((MB * NU, MB * NY), dtype=f)
    for j in range(MB):
        for i in range(MB):
            if i < j:
                Mji = (Cz * lam_p[j - 1 - i][None, :]) @ Bz   # (NY, NU)
                WU[i * NU : (i + 1) * NU, j * NY : (j + 1) * NY] = Mji.T
            elif i == j:
                WU[i * NU : (i + 1) * NU, j * NY : (j + 1) * NY] = D.T.astype(f)

    blocks = []
    for h in range(2):          # W2 order [g0h0, g1h0, g0h1, g1h1]
        for g in range(2):
            blocks.append(W2[128 * g : 128 * (g + 1), 128 * h : 128 * (h + 1)])
    for h in range(2):          # WC[h][g]
        for g in range(2):
            blocks.append(WC[128 * h : 128 * (h + 1), 128 * g : 128 * (g + 1)])
    # WU[g2][g] blocks; WU[1][0] is identically zero (i > j) and skipped
    blocks.append(WU[0:128, 0:128])      # WU00
    blocks.append(WU[0:128, 128:256])    # WU01
    blocks.append(WU[128:256, 128:256])  # WU11
    wAll = np.concatenate(blocks, axis=1).astype(bfd)     # (128, 11*128)

    # uT8[b, (i*32+u), k] = u[b, 8k+i, u]
    uT8 = np.ascontiguousarray(
        u.reshape(B, KCOL, MB, NU).transpose(0, 2, 3, 1).reshape(B, MB * NU, KCOL)
    ).astype(bfd)

    lam8 = lam**MB
    lam8c = np.stack([lam8[:128], lam8[128:]], axis=1).astype(f)  # (128, 2)
    return wAll, z0, uT8, lam8c


def make_in_maps(x0, u, Q, lam, Bmat, C, D):
    wAll, z0, uT8, lam8c = _host_prep(x0, u, Q, lam, Bmat, C, D)
    QL = L // 2
    in_maps = []
    for cidx in range(NCORES):
        sl = slice(cidx * BPC, (cidx + 1) * BPC)
        z0_c = z0[sl]
        z0c = z0_c.reshape(BPC, 2, 128).transpose(2, 0, 1).reshape(128, 2 * BPC)
        pz = np.ascontiguousarray(np.concatenate([lam8c, z0c], axis=1))
        ut = uT8[sl]  # (BPC, 256, KCOL)
        # uA[g, q]: quarters of b0 ch0; uB[j, g]: halves (b1c0, b0c1, b1c1)
        uA = np.ascontiguousarray(
            ut[0].reshape(2, 128, KCOL)[:, :, 0:L]
            .reshape(2, 128, 2, QL).transpose(0, 2, 1, 3)
        )
        ub_parts = []
        for (bb, ch) in [(1, 0), (0, 1), (1, 1)]:
            ub_parts.append(
                ut[bb].reshape(2, 128, KCOL)[:, :, ch * L : (ch + 1) * L]
            )
        uB = np.ascontiguousarray(np.stack(ub_parts, axis=0))
        in_maps.append(
            {
                "uA": uA,
                "uB": uB,
                "wAll": wAll,
                "pz": pz,
            }
        )
    return in_maps


def kernel(x0, u, Q, lam, Bmat, C, D):
    global _PROG
    from concourse.bass_utils import run_bass_kernel_spmd

    if _PROG is None:
        _PROG = build_program()
    in_maps = make_in_maps(x0, u, Q, lam, Bmat, C, D)
    res = run_bass_kernel_spmd(_PROG, in_maps, list(range(NCORES)))
    y = np.empty((B, T, NY), dtype=np.float32)
    for cidx in range(NCORES):
        yT8s_c = res.results[cidx]["yT8s"].astype(np.float32)  # (BPC, 2, 256, L)
        # y[b, 8*(ch*L+k)+j, yy] = yT8s[b, ch, 32j+yy, k]
        y[cidx * BPC : (cidx + 1) * BPC] = (
            yT8s_c.reshape(BPC, 2, MB, NY, L)
            .transpose(0, 1, 4, 2, 3)
            .reshape(BPC, T, NY)
        )
    return y



# revision 26
# speedup vs baseline: 1.1333x; 1.0774x over previous
"""Diagonalizable linear plant (modal state-space scan) on 8 Trainium2 cores.

y[b,t] = Cz @ z[b,t-1] + D @ u[b,t],  z[b,t] = lam * z[b,t-1] + Bz @ u[b,t]
with z[b,-1] = z0[b] = x0[b] @ Q, Bz = Q^T Bmat, Cz = C Q.

Sharding: data-parallel over batch (16 batches -> 2 per core).

Block-8 formulation (the DVE scan instruction runs at ~2 cycles/element,
so the time axis is decimated 8x before it reaches the scan; everything
else is full 128x128xN=512 bf16 matmuls, fp32 PSUM):
  host packs u as uT8[(i*32+u), k] = u[8k+i, u]        (256 rows = 2 K-groups)
  PE   V_h = W2^T @ U          W2[(i,u),n] = lam_n^(7-i) Bz[n,u]
  DVE  zB = scan(lam^8, V)     block-boundary states z_{8k+7}
  PE   Y_g = WC^T @ zBprev + WU^T @ U     (g indexes (j,y) output groups)
       WC[n,(j,y)] = lam_n^j Cz[y,n]
       WU[(i,u),(j,y)] = (Cz lam^(j-1-i) Bz)[y,u] for i<j, D[y,u] for i=j, else 0
  host unpacks yT8[(32j+y), k] -> y[8k+j, y]
"""

import numpy as np

B, T, NX, NU, NY = 16, 8192, 256, 32, 32
NCORES = 8
BPC = B // NCORES   # batches per core
MB = 8              # time-block folded into matmul K
KCOL = T // MB      # block columns per batch (1024)
L = 512             # block-columns per chunk
NCHUNK = KCOL // L  # chunks per batch (2)

_PROG = None  # built Bass program, cached across kernel() calls


def _patch_tile_drain():
    """walrus codegen in this container rejects >1 sync wait on one SP
    TPB_CTRL instruction (terminal TileContext drain / NoOp). Split the
    drain's waits across preceding SP nops carrying one wait each."""
    import concourse.tile as tile
    import concourse.mybir as mybir
    from concourse.vector_clock import ScopedClock

    if getattr(tile.TileContext, "_drain_patched", False):
        return

    def _drain_and_barrier(self, tick_clock, wait_clock):
        nc = self.nc
        scratch = nc.sync.nop()
        wait_clock.add_sem_waits(
            scratch.ins, ScopedClock({None: tick_clock.global_clock})
        )
        si = scratch.ins.sync_info
        waits = list(si.on_wait) if si is not None else []
        scratch.ins.sync_info = mybir.SyncInfo(on_wait=waits[:1], on_update=[])
        for w in waits[1:]:
            n2 = nc.sync.nop()
            n2.ins.sync_info = mybir.SyncInfo(on_wait=[w], on_update=[])
        nc.sync.drain()
        nc.all_engine_barrier()
        assert self.sems is not None
        popped = nc._tile_sem_poison_stack.pop()
        assert popped is self._sem_poison
        nc.clear_and_free_semaphores(list(self.sems.allocated().values()))
        nc.all_engine_barrier()

    tile.TileContext._drain_and_barrier = _drain_and_barrier
    tile.TileContext._drain_patched = True


def _split_multi_waits(nc, mybir):
    """This container's walrus codegen accepts at most ONE sync wait per
    instruction. Hoist extra waits into standalone EventSemaphore nops on
    the same engine, placed immediately before the instruction."""
    ctr = [0]

    def fresh(engine, wait):
        ctr[0] += 1
        ev = mybir.InstEventSemaphore(name=f"I-wsplit-{ctr[0]}", ins=[], outs=[])
        ev.engine = engine
        ev.sync_info = mybir.SyncInfo(on_wait=[wait], on_update=[])
        nc.register_instruction(ev)
        return ev

    for fn in nc.m.functions:
        for bb in fn.blocks:
            out = []
            changed = False
            for inst in bb.instructions:
                si = inst.sync_info
                waits = list(si.on_wait) if si is not None else []
                if len(waits) > 1:
                    changed = True
                    for w in waits[:-1]:
                        out.append(fresh(inst.engine, w))
                    inst.sync_info = mybir.SyncInfo(
                        on_wait=[waits[-1]], on_update=list(si.on_update)
                    )
                out.append(inst)
            if changed:
                bb.instructions = out


def build_program():
    import concourse.bass as bass
    import concourse.tile as tile
    import concourse.mybir as mybir
    from contextlib import ExitStack

    _patch_tile_drain()
    f32 = mybir.dt.float32
    bf = mybir.dt.bfloat16

    nc = bass.Bass()
    # uH[j, g, row, k]: half-chunks in unit-consumption order
    # j: (b0 ch0), (b1 ch0), (b0 ch1), (b1 ch1)   (128 KB contiguous each)
    uH = nc.declare_dram_parameter("uH", [4, 2, 128, L], bf, isOutput=False)
    wAll = nc.declare_dram_parameter("wAll", [128, 11 * 128], bf, isOutput=False)
    # pz: col 0:2 = lam^8 halves, cols 2: = z0 modal states (merged tiny DMA)
    pz = nc.declare_dram_parameter("pz", [128, 2 + 2 * BPC], f32, isOutput=False)
    yT8s = nc.declare_dram_parameter("yT8s", [BPC, 2, 256, L], bf, isOutput=True)

    with ExitStack() as ctx:
        tc = ctx.enter_context(tile.TileContext(nc))
        const = ctx.enter_context(tc.tile_pool(name="const", bufs=1))
        vps = ctx.enter_context(tc.tile_pool(name="vps", bufs=2, space="PSUM"))
        yps = ctx.enter_context(tc.tile_pool(name="yps", bufs=2, space="PSUM"))
        zpool = ctx.enter_context(tc.tile_pool(name="z", bufs=6))
        yout = ctx.enter_context(tc.tile_pool(name="yo", bufs=4))

        # DMA plan: scalar queue carries all g=0 (rows 0:128) U tiles,
        # sync all g=1, both in unit-consumption order; weights ride the
        # gpsimd software queue (W2 first, then WC, WU for the Y phase).
        pzt = const.tile([128, 2 + 2 * BPC], f32)
        nc.sync.dma_start(pzt[:], pz[:])
        W2t = const.tile([128, 512], bf)
        nc.gpsimd.dma_start(W2t[:], wAll[:, 0:512])
        WCt = const.tile([128, 512], bf)
        nc.gpsimd.dma_start(WCt[:], wAll[:, 512:1024])
        WUt = const.tile([128, 384], bf)
        nc.gpsimd.dma_start(WUt[:], wAll[:, 1024:1408])
        lam8t = pzt[:, 0:2]
        z0t = pzt[:, 2 : 2 + 2 * BPC]
        Ubig = [[const.tile([128, KCOL], bf, name=f"U{g}b{b}") for g in range(2)]
                for b in range(BPC)]
        qeng = [nc.scalar, nc.sync]
        for j, (bb, ch) in enumerate([(0, 0), (1, 0), (0, 1), (1, 1)]):
            for g in range(2):
                qeng[g].dma_start(Ubig[bb][g][:, ch * L : (ch + 1) * L],
                                  uH[j, g, :, :])

        # PE warm-up during the DMA fill window
        dummy = const.tile([128, L], bf)
        nc.vector.memset(dummy[:], 0.0)
        WP = vps.tile([128, L], f32, name="WP", tag="V0")
        for _ in range(3):
            nc.tensor.matmul(WP[:], lhsT=dummy[:, 0:128], rhs=dummy[:],
                             start=True, stop=True)

        # lam broadcast built on DVE during the DMA fill window
        ones = const.tile([128, L], f32)
        nc.vector.memset(ones[:], 1.0)
        lam_bc = const.tile([128, 2 * L], f32)
        for h in range(2):
            nc.vector.tensor_scalar_mul(
                lam_bc[:, h * L : (h + 1) * L], ones[:], lam8t[:, h : h + 1]
            )

        def w2blk(i):
            return W2t[:, 128 * i : 128 * (i + 1)]

        def wcblk(i):
            return WCt[:, 128 * i : 128 * (i + 1)]

        # wAll W2 block order: [g0h0, g1h0, g0h1, g1h1] (h=0 pair first)
        W2 = [[w2blk(0), w2blk(2)], [w2blk(1), w2blk(3)]]      # [g][h]
        WC = [[wcblk(0), wcblk(1)], [wcblk(2), wcblk(3)]]      # [h][g]
        WU00 = WUt[:, 0:128]
        WU01 = WUt[:, 128:256]
        WU11 = WUt[:, 256:384]                                 # WU[1][0] == 0

        mult = mybir.AluOpType.mult
        add = mybir.AluOpType.add

        prev_z = [[None, None] for _ in range(BPC)]

        def emit_vscan(c, b):
            sl = slice(c * L, (c + 1) * L)
            U = [Ubig[b][0][:, sl], Ubig[b][1][:, sl]]
            zext = [None, None]
            for h in range(2):
                V = vps.tile([128, L], f32, name=f"V{h}_{b}_{c}", tag=f"V{h}")
                Z = zpool.tile([128, L + 1], bf, name=f"Z{h}_{b}_{c}",
                               tag=f"Z{h}")
                carry = (z0t[:, 2 * b + h : 2 * b + h + 1] if c == 0
                         else prev_z[b][h][:, L : L + 1])
                nc.tensor.matmul(V[:], lhsT=W2[0][h], rhs=U[0],
                                 start=True, stop=False)
                nc.tensor.matmul(V[:], lhsT=W2[1][h], rhs=U[1],
                                 start=False, stop=True)
                nc.vector.tensor_tensor_scan(
                    Z[:, 1 : L + 1], lam_bc[:, h * L : (h + 1) * L], V[:],
                    carry, mult, add,
                )
                nc.gpsimd.tensor_copy(Z[:, 0:1], carry)
                zext[h] = Z
            prev_z[b] = zext
            return U, zext

        def emit_y(c, b, U, zext, last=False):
            for g in range(2):
                Y = yps.tile([128, L], f32, name=f"Y{g}_{b}_{c}", tag=f"Y{g}")
                wu = ([(WU00, U[0])] if g == 0
                      else [(WU01, U[0]), (WU11, U[1])])
                wc = [(WC[0][g], zext[0][:, 0:L]), (WC[1][g], zext[1][:, 0:L])]
                # early units: WC weights arrive before WU; late units: WU
                # is long loaded and scan h1 lands last, so WU goes first
                mms = wc + wu if c == 0 else wu + wc
                for i, (lhsT, rhs) in enumerate(mms):
                    nc.tensor.matmul(Y[:], lhsT=lhsT, rhs=rhs,
                                     start=(i == 0), stop=(i == len(mms) - 1))
                Ysb = yout.tile([128, L], bf, name=f"Ysb{g}_{b}_{c}",
                                tag=f"Ysb{g}")
                oeng = nc.sync if g == 0 else nc.scalar
                dst = yT8s[b, c, 128 * g : 128 * (g + 1), :]
                if not last:
                    nc.scalar.copy(Ysb[:], Y[:])
                    oeng.dma_start(dst, Ysb[:])
                else:
                    # tail: halves; g0 copies on vector (free after the last
                    # scan), g1 on scalar, stores fan out to both queues
                    H = L // 2
                    for p in range(2):
                        s = slice(p * H, (p + 1) * H)
                        if g == 0:
                            nc.vector.tensor_copy(Ysb[:, s], Y[:, s])
                        else:
                            nc.scalar.copy(Ysb[:, s], Y[:, s])
                        oeng.dma_start(dst[:, s], Ysb[:, s])

        units = [(c, b) for c in range(NCHUNK) for b in range(BPC)]
        pending = []
        for (c, b) in units:
            U, zext = emit_vscan(c, b)
            pending.append((c, b, U, zext))
            if len(pending) > 2:
                emit_y(*pending.pop(0))
        for i, p in enumerate(pending):
            emit_y(*p, last=(i == len(pending) - 1))

    _split_multi_waits(nc, mybir)
    return nc


def _host_prep(x0, u, Q, lam, Bmat, C, D):
    import ml_dtypes

    f = np.float32
    bfd = ml_dtypes.bfloat16
    lam = lam.astype(f)
    Bz = (Q.T.astype(f) @ Bmat.astype(f)).astype(f)      # (NX, NU)
    Cz = (C.astype(f) @ Q.astype(f)).astype(f)           # (NY, NX)
    z0 = (x0.astype(f) @ Q.astype(f)).astype(f)          # (B, NX)

    lam_p = np.stack([lam**j for j in range(MB)])         # (MB, NX)

    # W2[(i*32+u), n] = lam_n^(MB-1-i) * Bz[n, u]
    W2 = np.einsum("in,nu->iun", lam_p[::-1], Bz).reshape(MB * NU, NX)
    # WC[n, (32j+y)] = lam_n^j * Cz[y, n]
    WC = np.einsum("jn,yn->njy", lam_p, Cz).reshape(NX, MB * NY)
    # WU[(i*32+u), (32j+y)]
    WU = np.zeros((MB * NU, MB * NY), dtype=f)
    for j in range(MB):
        for i in range(MB):
            if i < j:
                Mji = (Cz * lam_p[j - 1 - i][None, :]) @ Bz   # (NY, NU)
                WU[i * NU : (i + 1) * NU, j * NY : (j + 1) * NY] = Mji.T
            elif i == j:
                WU[i * NU : (i + 1) * NU, j * NY : (j + 1) * NY] = D.T.astype(f)

    blocks = []
    for h in range(2):          # W2 order [g0h0, g1h0, g0h1, g1h1]
        for g in range(2):
            blocks.append(W2[128 * g : 128 * (g + 1), 128 * h : 128 * (h + 1)])
    for h in range(2):          # WC[h][g]
        for g in range(2):
            blocks.append(WC[128 * h : 128 * (h + 1), 128 * g : 128 * (g + 1)])
    # WU[g2][g] blocks; WU[1][0] is identically zero (i > j) and skipped
    blocks.append(WU[0:128, 0:128])      # WU00
    blocks.append(WU[0:128, 128:256])    # WU01
    blocks.append(WU[128:256, 128:256])  # WU11
    wAll = np.concatenate(blocks, axis=1).astype(bfd)     # (128, 11*128)

    # uT8[b, (i*32+u), k] = u[b, 8k+i, u]
    uT8 = np.ascontiguousarray(
        u.reshape(B, KCOL, MB, NU).transpose(0, 2, 3, 1).reshape(B, MB * NU, KCOL)
    ).astype(bfd)

    lam8 = lam**MB
    lam8c = np.stack([lam8[:128], lam8[128:]], axis=1).astype(f)  # (128, 2)
    return wAll, z0, uT8, lam8c


def make_in_maps(x0, u, Q, lam, Bmat, C, D):
    wAll, z0, uT8, lam8c = _host_prep(x0, u, Q, lam, Bmat, C, D)
    in_maps = []
    for cidx in range(NCORES):
        sl = slice(cidx * BPC, (cidx + 1) * BPC)
        z0_c = z0[sl]
        z0c = z0_c.reshape(BPC, 2, 128).transpose(2, 0, 1).reshape(128, 2 * BPC)
        pz = np.ascontiguousarray(np.concatenate([lam8c, z0c], axis=1))
        ut = uT8[sl]  # (BPC, 256, KCOL)
        uh_parts = []
        for (bb, ch) in [(0, 0), (1, 0), (0, 1), (1, 1)]:
            uh_parts.append(
                ut[bb].reshape(2, 128, KCOL)[:, :, ch * L : (ch + 1) * L]
            )
        uH = np.ascontiguousarray(np.stack(uh_parts, axis=0))
        in_maps.append(
            {
                "uH": uH,
                "wAll": wAll,
                "pz": pz,
            }
        )
    return in_maps


def kernel(x0, u, Q, lam, Bmat, C, D):
    global _PROG
    from concourse.bass_utils import run_bass_kernel_spmd

    if _PROG is None:
        _PROG = build_program()
    in_maps = make_in_maps(x0, u, Q, lam, Bmat, C, D)
    res = run_bass_kernel_spmd(_PROG, in_maps, list(range(NCORES)))
    y = np.empty((B, T, NY), dtype=np.float32)
    for cidx in range(NCORES):
        yT8s_c = res.results[cidx]["yT8s"].astype(np.float32)  # (BPC, 2, 256, L)
        # y[b, 8*(ch*L+k)+j, yy] = yT8s[b, ch, 32j+yy, k]
        y[cidx * BPC : (cidx + 1) * BPC] = (
            yT8s_c.reshape(BPC, 2, MB, NY, L)
            .transpose(0, 1, 4, 2, 3)
            .reshape(BPC, T, NY)
        )
    return y

